# revision 14
# baseline (speedup 1.0000x reference)
"""Trainium2 Bass kernel for nn_AttentionPool (segment softmax-pool over gene/spot edges).

Math: out[g] = (sum_{s in S_g} e_s * emb[s]) / (sum_{s in S_g} e_s),
      e_s = exp(logit_s - 30),  logit = tanh(emb @ W.T + b) @ v
where S_g is the *set* of distinct spots expressing gene g (duplicate edges
count once), and empty genes produce 0. The row-max shift of the reference
softmax cancels; the constant -30 shift keeps exp() in fp32 range
(|logit| <= sum|v| < 27.6 for this problem's xavier init).

Sharding: 2500 genes per core x 8 cores (padded to 2560 = 20 tiles of 128).
Host marshals the edge list into each core's dense 0/1 mask slab, laid out
as [20 gene-tiles, 128 spot-partition, 32 spot-chunk, 128 gene] bf16 so each
strip is one contiguous 1MB DMA and each [128,128] chunk is a matmul lhsT.
All floating-point math runs on device. X is carried as bf16 hi+lo pairs so
the bf16 matmuls reproduce fp32 accuracy (~1e-6).
"""

import sys

sys.path.insert(0, "/opt/trn_rl_repo")

import numpy as np
import ml_dtypes

import concourse.mybir as mybir
import concourse.tile as tile
from concourse import bacc
from concourse.bass import ts
from concourse.bass_utils import run_bass_kernel_spmd
from concourse.bass_interp import get_hw_module

F32 = mybir.dt.float32
BF16 = mybir.dt.bfloat16
U8 = mybir.dt.uint8

N_SPOTS = 4096
N_GENES = 20000
D = 128
N_CORES = 8
G_PER = N_GENES // N_CORES  # 2500
P = 128
KCH = N_SPOTS // P  # 32 spot chunks
NX = 258  # [Xhi | Xlo] columns: 2 * (D + 1)


def build_nc(T, wide_mm=False):
    """Build the single-core Bass program (SPMD across 8 cores).

    T = number of 128-gene tiles per core (20 for the real problem).
    wide_mm = one N=258 matmul per chunk (single LDWEIGHTS) instead of two
    N=129 matmuls sharing PSUM columns.
    """
    nc = bacc.Bacc("TRN2", target_bir_lowering=False, debug=False, num_devices=N_CORES)

    maskbt = nc.dram_tensor("maskbt", [T, P, KCH * P], U8, kind="ExternalInput")
    embc = nc.dram_tensor("embc", [KCH, P, D], F32, kind="ExternalInput")
    embT = nc.dram_tensor("embT", [D, N_SPOTS], F32, kind="ExternalInput")
    wt = nc.dram_tensor("wt", [D, D], F32, kind="ExternalInput")
    bb = nc.dram_tensor("bb", [D, 1], F32, kind="ExternalInput")
    vv = nc.dram_tensor("vv", [D, 1], F32, kind="ExternalInput")
    out = nc.dram_tensor("out", [T, P, D], F32, kind="ExternalOutput")

    with tile.TileContext(nc) as tc:
        with (
            tc.tile_pool(name="const", bufs=1) as constp,
            tc.tile_pool(name="xfp", bufs=1) as xfp,
            tc.tile_pool(name="maskp", bufs=4) as maskp,
            tc.tile_pool(name="outp", bufs=2) as outp,
            tc.tile_pool(name="php", bufs=4, space="PSUM") as php,
            tc.tile_pool(name="pep", bufs=1, space="PSUM") as pep,
            tc.tile_pool(name="ptp", bufs=3, space="PSUM") as ptp,
        ):
            # ---- constants into SBUF ----
            wt_sb = constp.tile([P, D], F32)
            nc.sync.dma_start(out=wt_sb[:], in_=wt[:])
            b_sb = constp.tile([P, 1], F32)
            nc.sync.dma_start(out=b_sb[:], in_=bb[:])
            v_sb = constp.tile([P, 1], F32)
            nc.sync.dma_start(out=v_sb[:], in_=vv[:])
            embT_sb = constp.tile([P, N_SPOTS], F32)
            nc.sync.dma_start(out=embT_sb[:], in_=embT[:])

            neg30 = constp.tile([P, 1], F32)
            nc.gpsimd.memset(neg30[:], -30.0)

            th_sb = constp.tile([P, N_SPOTS], F32)  # tanh(W h + b).T  [j, s]
            e_sb = constp.tile([P, KCH], F32)  # e in spot-partition layout
            xhl = constp.tile([P, KCH * NX], BF16)  # [Xhi | Xlo] per chunk

            # ---- prologue: logits ----
            # h.T [j, s] = (W.T).T @ emb.T ; tanh(+b) fused from PSUM
            for c in range(N_SPOTS // 512):
                ph = php.tile([P, 512], F32)
                nc.tensor.matmul(
                    out=ph[:], lhsT=wt_sb[:], rhs=embT_sb[:, ts(c, 512)],
                    start=True, stop=True,
                )
                nc.scalar.activation(
                    out=th_sb[:, ts(c, 512)], in_=ph[:],
                    func=mybir.ActivationFunctionType.Tanh, bias=b_sb[:, 0:1],
                )
            # logits, transposed into spot-partition layout:
            # logitsT chunk [128 s, 1] = th_chunk[j, s].T @ v
            pe = pep.tile([P, KCH], F32)
            for k in range(KCH):
                nc.tensor.matmul(
                    out=pe[:, k : k + 1], lhsT=th_sb[:, ts(k, P)], rhs=v_sb[:],
                    start=True, stop=True,
                )
            nc.scalar.activation(
                out=e_sb[:], in_=pe[:],
                func=mybir.ActivationFunctionType.Exp, bias=neg30[:, 0:1],
            )

            # ---- X = [e*emb | e] as bf16 hi + lo (full-width batched ops) ----
            # emb in spot-partition layout, all 32 chunks in one DMA
            embc_sb = constp.tile([P, KCH * D], F32)
            nc.sync.dma_start(
                out=embc_sb[:].rearrange("p (k d) -> p k d", d=D),
                in_=embc[:].rearrange("k p d -> p k d"),
            )
            xf = xfp.tile([P, KCH * D], F32)
            xhl3 = xhl[:].rearrange("p (k n) -> p k n", n=NX)
            xf3 = xf[:].rearrange("p (k d) -> p k d", d=D)
            emb3 = embc_sb[:].rearrange("p (k d) -> p k d", d=D)
            e3 = e_sb[:].rearrange("p k -> p k ()")
            NG = 4  # build X in chunk groups so the main loop starts early
            GS = KCH // NG
            for g in range(NG):
                ks = slice(g * GS, (g + 1) * GS)
                ebc = e3[:, ks, :].to_broadcast([P, GS, D])
                nc.vector.tensor_mul(out=xf3[:, ks, :], in0=emb3[:, ks, :], in1=ebc)
                hi3 = xhl3[:, ks, 0:D]
                nc.scalar.activation(
                    out=hi3, in_=xf3[:, ks, :], func=mybir.ActivationFunctionType.Copy
                )
                nc.vector.tensor_sub(
                    out=xhl3[:, ks, D + 1 : NX - 1], in0=xf3[:, ks, :], in1=hi3
                )
                nc.vector.tensor_copy(out=xhl3[:, ks, D : D + 1], in_=e3[:, ks, :])
                nc.vector.tensor_sub(
                    out=xhl3[:, ks, NX - 1 : NX], in0=e3[:, ks, :],
                    in1=xhl3[:, ks, D : D + 1],
                )

            # ---- main loop: per gene tile ----
            for t in range(T):
                mt = maskp.tile([P, KCH * P], BF16)
                nc.gpsimd.dma_start(out=mt[:], in_=maskbt[t])  # uint8 -> bf16 cast in DMA
                if wide_mm:
                    # one LDW + one N=258 matmul per chunk; hi/lo halves
                    # summed on DVE afterwards
                    pt = ptp.tile([P, NX], F32)
                    for k in range(KCH):
                        nc.tensor.matmul(
                            out=pt[:], lhsT=mt[:, ts(k, P)], rhs=xhl[:, ts(k, NX)],
                            start=(k == 0), stop=(k == KCH - 1),
                        )
                    lo_sb = outp.tile([P, D + 1], F32)
                    nc.vector.tensor_copy(out=lo_sb[:], in_=pt[:, D + 1 : NX])
                    s_sb = outp.tile([P, D + 1], F32)
                    nc.vector.tensor_add(out=s_sb[:], in0=pt[:, 0 : D + 1], in1=lo_sb[:])
                else:
                    # hi and lo accumulate into the same PSUM columns
                    pt = ptp.tile([P, D + 1], F32)
                    for k in range(KCH):
                        nc.tensor.matmul(
                            out=pt[:], lhsT=mt[:, ts(k, P)],
                            rhs=xhl[:, k * NX : k * NX + (D + 1)],
                            start=(k == 0), stop=False,
                        )
                        nc.tensor.matmul(
                            out=pt[:], lhsT=mt[:, ts(k, P)],
                            rhs=xhl[:, k * NX + (D + 1) : (k + 1) * NX],
                            start=False, stop=(k == KCH - 1),
                        )
                    s_sb = pt
                rmax = outp.tile([P, 1], F32)
                nc.vector.tensor_scalar_max(out=rmax[:], in0=s_sb[:, D : D + 1], scalar1=1e-37)
                rinv = outp.tile([P, 1], F32)
                nc.vector.reciprocal(out=rinv[:], in_=rmax[:])
                o = outp.tile([P, D], F32)
                nc.vector.tensor_scalar_mul(out=o[:], in0=s_sb[:, 0:D], scalar1=rinv[:, 0:1])
                nc.sync.dma_start(out=out[t], in_=o[:])

    nc.compile()
    return nc


def prep_inputs(spot_emb, W, b, v, gene_ids, spot_ids, T):
    """Host marshaling: shared fp32 operands + per-core mask slabs."""
    emb = np.ascontiguousarray(np.asarray(spot_emb, dtype=np.float32))
    W = np.asarray(W, dtype=np.float32)
    b = np.asarray(b, dtype=np.float32)
    v = np.asarray(v, dtype=np.float32)
    gene_ids = np.asarray(gene_ids).astype(np.int64)
    spot_ids = np.asarray(spot_ids).astype(np.int64)

    shared = {
        "embc": np.ascontiguousarray(emb.reshape(KCH, P, D)),
        "embT": np.ascontiguousarray(emb.T),
        "wt": np.ascontiguousarray(W.T),
        "bb": np.ascontiguousarray(b.reshape(D, 1)),
        "vv": np.ascontiguousarray(v.reshape(D, 1)),
    }

    # Dense 0/1 occupancy mask (set semantics: duplicate edges collapse),
    # built directly in the per-core padded layout: core c's genes live at
    # rows [c*T*P, c*T*P + G_PER); rows above G_PER stay zero padding.
    g_pad = T * P
    M = np.zeros((N_CORES * g_pad, N_SPOTS), dtype=bool)
    pad_rows = (gene_ids // G_PER) * g_pad + (gene_ids % G_PER)
    M[pad_rows, spot_ids] = True
    # [c, t*128+g, k*128+p] -> [c, t, p, k, g]
    Mbt = M.reshape(N_CORES, T, P, KCH, P).transpose(0, 1, 4, 3, 2)
    Mbt = np.ascontiguousarray(Mbt).astype(np.uint8).reshape(N_CORES, T, P, KCH * P)
    return [{"maskbt": Mbt[c], **shared} for c in range(N_CORES)]


_NC_CACHE = {}


def run(spot_emb, W, b, v, gene_ids, spot_ids, trace=False, wide_mm=False, **hw_kwargs):
    T = (G_PER + P - 1) // P  # 20
    key = (T, wide_mm)
    if key not in _NC_CACHE:
        nc = build_nc(T, wide_mm=wide_mm)
        nc.m = get_hw_module(nc.m)
        _NC_CACHE[key] = nc
    nc = _NC_CACHE[key]
    in_maps = prep_inputs(spot_emb, W, b, v, gene_ids, spot_ids, T)
    res = run_bass_kernel_spmd(
        nc, in_maps, core_ids=list(range(N_CORES)), trace=trace, **hw_kwargs
    )
    outs = [
        np.asarray(res.results[c]["out"], dtype=np.float32).reshape(T * P, D)[:G_PER]
        for c in range(N_CORES)
    ]
    full = np.concatenate(outs, axis=0)
    return full, res


def kernel(spot_emb, W, b, v, gene_ids, spot_ids, n_genes):
    n_genes = int(n_genes)
    assert n_genes == N_GENES, f"kernel hardcodes n_genes={N_GENES}, got {n_genes}"
    full, _ = run(spot_emb, W, b, v, gene_ids, spot_ids, trace=False)
    return full


# revision 21
# speedup vs baseline: 1.0311x; 1.0311x over previous
"""Trainium2 Bass kernel for nn_AttentionPool (segment softmax-pool over gene/spot edges).

Math: out[g] = (sum_{s in S_g} e_s * emb[s]) / (sum_{s in S_g} e_s),
      e_s = exp(logit_s - 30),  logit = tanh(emb @ W.T + b) @ v
where S_g is the *set* of distinct spots expressing gene g (duplicate edges
count once), and empty genes produce 0. The row-max shift of the reference
softmax cancels; the constant -30 shift keeps exp() in fp32 range
(|logit| <= sum|v| < 27.6 for this problem's xavier init).

Sharding: 2500 genes per core x 8 cores (padded to 2560 = 20 tiles of 128).
Host marshals the edge list into each core's dense 0/1 mask slab, laid out
as [20 gene-tiles, 128 spot-partition, 32 spot-chunk, 128 gene] bf16 so each
strip is one contiguous 1MB DMA and each [128,128] chunk is a matmul lhsT.
All floating-point math runs on device. X is carried as bf16 hi+lo pairs so
the bf16 matmuls reproduce fp32 accuracy (~1e-6).
"""

import sys

sys.path.insert(0, "/opt/trn_rl_repo")

import numpy as np
import ml_dtypes

import concourse.mybir as mybir
import concourse.tile as tile
from concourse import bacc
from concourse.bass import ts
from concourse.tile import add_dep_helper
from concourse.bass_utils import run_bass_kernel_spmd
from concourse.bass_interp import get_hw_module

F32 = mybir.dt.float32
BF16 = mybir.dt.bfloat16
U8 = mybir.dt.uint8

N_SPOTS = 4096
N_GENES = 20000
D = 128
N_CORES = 8
G_PER = N_GENES // N_CORES  # 2500
P = 128
KCH = N_SPOTS // P  # 32 spot chunks
NX = 258  # [Xhi | Xlo] columns: 2 * (D + 1)


def build_nc(T, wide_mm=False):
    """Build the single-core Bass program (SPMD across 8 cores).

    T = number of 128-gene tiles per core (20 for the real problem).
    wide_mm = one N=258 matmul per chunk (single LDWEIGHTS) instead of two
    N=129 matmuls sharing PSUM columns.
    """
    nc = bacc.Bacc("TRN2", target_bir_lowering=False, debug=False, num_devices=N_CORES)

    maskbt = nc.dram_tensor("maskbt", [T, P, KCH * P], U8, kind="ExternalInput")
    # emb pre-swizzled on host to spot-partition layout: [p, k*128+d] =
    # emb[k*128+p, d] -> each SBUF partition line is one contiguous 16KB read
    embcp = nc.dram_tensor("embcp", [P, KCH * D], F32, kind="ExternalInput")
    embT = nc.dram_tensor("embT", [D, N_SPOTS], F32, kind="ExternalInput")
    wt = nc.dram_tensor("wt", [D, D], F32, kind="ExternalInput")
    bb = nc.dram_tensor("bb", [D, 1], F32, kind="ExternalInput")
    vv = nc.dram_tensor("vv", [D, 1], F32, kind="ExternalInput")
    out = nc.dram_tensor("out", [T, P, D], F32, kind="ExternalOutput")

    with tile.TileContext(nc) as tc:
        with (
            tc.tile_pool(name="const", bufs=1) as constp,
            tc.tile_pool(name="xfp", bufs=1) as xfp,
            tc.tile_pool(name="maskp", bufs=4) as maskp,
            tc.tile_pool(name="outp", bufs=2) as outp,
            tc.tile_pool(name="php", bufs=4, space="PSUM") as php,
            tc.tile_pool(name="pep", bufs=1, space="PSUM") as pep,
            tc.tile_pool(name="ptp", bufs=3, space="PSUM") as ptp,
        ):
            # ---- constants into SBUF ----
            wt_sb = constp.tile([P, D], F32)
            nc.sync.dma_start(out=wt_sb[:], in_=wt[:])
            b_sb = constp.tile([P, 1], F32)
            nc.sync.dma_start(out=b_sb[:], in_=bb[:])
            v_sb = constp.tile([P, 1], F32)
            nc.sync.dma_start(out=v_sb[:], in_=vv[:])
            embT_sb = constp.tile([P, N_SPOTS], F32)
            embT_dma = nc.sync.dma_start(out=embT_sb[:], in_=embT[:])

            neg30 = constp.tile([P, 1], F32)
            nc.gpsimd.memset(neg30[:], -30.0)

            th_sb = constp.tile([P, N_SPOTS], F32)  # tanh(W h + b).T  [j, s]
            e_sb = constp.tile([P, KCH], F32)  # e in spot-partition layout
            xhl = constp.tile([P, KCH * NX], BF16)  # [Xhi | Xlo] per chunk

            # ---- prologue: logits ----
            # h.T [j, s] = (W.T).T @ emb.T ; tanh(+b) fused from PSUM
            for c in range(N_SPOTS // 512):
                ph = php.tile([P, 512], F32)
                nc.tensor.matmul(
                    out=ph[:], lhsT=wt_sb[:], rhs=embT_sb[:, ts(c, 512)],
                    start=True, stop=True,
                )
                nc.scalar.activation(
                    out=th_sb[:, ts(c, 512)], in_=ph[:],
                    func=mybir.ActivationFunctionType.Tanh, bias=b_sb[:, 0:1],
                )
            # logits, transposed into spot-partition layout:
            # logitsT chunk [128 s, 1] = th_chunk[j, s].T @ v
            pe = pep.tile([P, KCH], F32)
            for k in range(KCH):
                nc.tensor.matmul(
                    out=pe[:, k : k + 1], lhsT=th_sb[:, ts(k, P)], rhs=v_sb[:],
                    start=True, stop=True,
                )
            nc.scalar.activation(
                out=e_sb[:], in_=pe[:],
                func=mybir.ActivationFunctionType.Exp, bias=neg30[:, 0:1],
            )

            # ---- X = [e*emb | e] as bf16 hi + lo (full-width batched ops) ----
            # emb in spot-partition layout; scalar HWDGE ring so it runs in
            # parallel with the embT load on the sync ring
            embc_sb = constp.tile([P, KCH * D], F32)
            embc_dma = nc.scalar.dma_start(out=embc_sb[:], in_=embcp[:])
            xf = xfp.tile([P, KCH * D], F32)
            xhl3 = xhl[:].rearrange("p (k n) -> p k n", n=NX)
            xf3 = xf[:].rearrange("p (k d) -> p k d", d=D)
            emb3 = embc_sb[:].rearrange("p (k d) -> p k d", d=D)
            e3 = e_sb[:].rearrange("p k -> p k ()")
            NG = 4  # build X in chunk groups so the main loop starts early
            GS = KCH // NG
            for g in range(NG):
                ks = slice(g * GS, (g + 1) * GS)
                ebc = e3[:, ks, :].to_broadcast([P, GS, D])
                nc.vector.tensor_mul(out=xf3[:, ks, :], in0=emb3[:, ks, :], in1=ebc)
                hi3 = xhl3[:, ks, 0:D]
                nc.scalar.activation(
                    out=hi3, in_=xf3[:, ks, :], func=mybir.ActivationFunctionType.Copy
                )
                nc.vector.tensor_sub(
                    out=xhl3[:, ks, D + 1 : NX - 1], in0=xf3[:, ks, :], in1=hi3
                )
                nc.vector.tensor_copy(out=xhl3[:, ks, D : D + 1], in_=e3[:, ks, :])
                nc.vector.tensor_sub(
                    out=xhl3[:, ks, NX - 1 : NX], in0=e3[:, ks, :],
                    in1=xhl3[:, ks, D : D + 1],
                )

            # ---- main loop: per gene tile ----
            for t in range(T):
                mt = maskp.tile([P, KCH * P], BF16)
                mdma = nc.gpsimd.dma_start(out=mt[:], in_=maskbt[t])  # uint8 -> bf16 cast in DMA
                if t < 4:
                    # keep the prefetch burst from stealing SDMA engines
                    # while the latency-critical emb loads are in flight
                    add_dep_helper(mdma.ins, embT_dma.ins, True, "mask prefetch after embT")
                    add_dep_helper(mdma.ins, embc_dma.ins, True, "mask prefetch after embc")
                if wide_mm:
                    # one LDW + one N=258 matmul per chunk; hi/lo halves
                    # summed on DVE afterwards
                    pt = ptp.tile([P, NX], F32)
                    for k in range(KCH):
                        nc.tensor.matmul(
                            out=pt[:], lhsT=mt[:, ts(k, P)], rhs=xhl[:, ts(k, NX)],
                            start=(k == 0), stop=(k == KCH - 1),
                        )
                    lo_sb = outp.tile([P, D + 1], F32)
                    nc.vector.tensor_copy(out=lo_sb[:], in_=pt[:, D + 1 : NX])
                    s_sb = outp.tile([P, D + 1], F32)
                    nc.vector.tensor_add(out=s_sb[:], in0=pt[:, 0 : D + 1], in1=lo_sb[:])
                else:
                    # hi and lo accumulate into the same PSUM columns
                    pt = ptp.tile([P, D + 1], F32)
                    for k in range(KCH):
                        nc.tensor.matmul(
                            out=pt[:], lhsT=mt[:, ts(k, P)],
                            rhs=xhl[:, k * NX : k * NX + (D + 1)],
                            start=(k == 0), stop=False,
                        )
                        nc.tensor.matmul(
                            out=pt[:], lhsT=mt[:, ts(k, P)],
                            rhs=xhl[:, k * NX + (D + 1) : (k + 1) * NX],
                            start=False, stop=(k == KCH - 1),
                        )
                    s_sb = pt
                rmax = outp.tile([P, 1], F32)
                nc.vector.tensor_scalar_max(out=rmax[:], in0=s_sb[:, D : D + 1], scalar1=1e-37)
                rinv = outp.tile([P, 1], F32)
                nc.vector.reciprocal(out=rinv[:], in_=rmax[:])
                o = outp.tile([P, D], F32)
                nc.vector.tensor_scalar_mul(out=o[:], in0=s_sb[:, 0:D], scalar1=rinv[:, 0:1])
                nc.sync.dma_start(out=out[t], in_=o[:])

    nc.compile()
    return nc


def prep_inputs(spot_emb, W, b, v, gene_ids, spot_ids, T):
    """Host marshaling: shared fp32 operands + per-core mask slabs."""
    emb = np.ascontiguousarray(np.asarray(spot_emb, dtype=np.float32))
    W = np.asarray(W, dtype=np.float32)
    b = np.asarray(b, dtype=np.float32)
    v = np.asarray(v, dtype=np.float32)
    gene_ids = np.asarray(gene_ids).astype(np.int64)
    spot_ids = np.asarray(spot_ids).astype(np.int64)

    shared = {
        "embcp": np.ascontiguousarray(
            emb.reshape(KCH, P, D).transpose(1, 0, 2).reshape(P, KCH * D)
        ),
        "embT": np.ascontiguousarray(emb.T),
        "wt": np.ascontiguousarray(W.T),
        "bb": np.ascontiguousarray(b.reshape(D, 1)),
        "vv": np.ascontiguousarray(v.reshape(D, 1)),
    }

    # Dense 0/1 occupancy mask (set semantics: duplicate edges collapse),
    # built directly in the per-core padded layout: core c's genes live at
    # rows [c*T*P, c*T*P + G_PER); rows above G_PER stay zero padding.
    g_pad = T * P
    M = np.zeros((N_CORES * g_pad, N_SPOTS), dtype=bool)
    pad_rows = (gene_ids // G_PER) * g_pad + (gene_ids % G_PER)
    M[pad_rows, spot_ids] = True
    # [c, t*128+g, k*128+p] -> [c, t, p, k, g]
    Mbt = M.reshape(N_CORES, T, P, KCH, P).transpose(0, 1, 4, 3, 2)
    Mbt = np.ascontiguousarray(Mbt).astype(np.uint8).reshape(N_CORES, T, P, KCH * P)
    return [{"maskbt": Mbt[c], **shared} for c in range(N_CORES)]


_NC_CACHE = {}


def run(spot_emb, W, b, v, gene_ids, spot_ids, trace=False, wide_mm=False, **hw_kwargs):
    T = (G_PER + P - 1) // P  # 20
    key = (T, wide_mm)
    if key not in _NC_CACHE:
        nc = build_nc(T, wide_mm=wide_mm)
        nc.m = get_hw_module(nc.m)
        _NC_CACHE[key] = nc
    nc = _NC_CACHE[key]
    in_maps = prep_inputs(spot_emb, W, b, v, gene_ids, spot_ids, T)
    res = run_bass_kernel_spmd(
        nc, in_maps, core_ids=list(range(N_CORES)), trace=trace, **hw_kwargs
    )
    outs = [
        np.asarray(res.results[c]["out"], dtype=np.float32).reshape(T * P, D)[:G_PER]
        for c in range(N_CORES)
    ]
    full = np.concatenate(outs, axis=0)
    return full, res


def kernel(spot_emb, W, b, v, gene_ids, spot_ids, n_genes):
    n_genes = int(n_genes)
    assert n_genes == N_GENES, f"kernel hardcodes n_genes={N_GENES}, got {n_genes}"
    full, _ = run(spot_emb, W, b, v, gene_ids, spot_ids, trace=False)
    return full


# revision 25
# speedup vs baseline: 1.0679x; 1.0357x over previous
"""Trainium2 Bass kernel for nn_AttentionPool (segment softmax-pool over gene/spot edges).

Math: out[g] = (sum_{s in S_g} e_s * emb[s]) / (sum_{s in S_g} e_s),
      e_s = exp(logit_s - 30),  logit = tanh(emb @ W.T + b) @ v
where S_g is the *set* of distinct spots expressing gene g (duplicate edges
count once), and empty genes produce 0. The row-max shift of the reference
softmax cancels; the constant -30 shift keeps exp() in fp32 range
(|logit| <= sum|v| < 27.6 for this problem's xavier init).

Sharding: 2500 genes per core x 8 cores (padded to 2560 = 20 tiles of 128).
Host marshals the edge list into each core's dense 0/1 mask slab, laid out
as [20 gene-tiles, 128 spot-partition, 32 spot-chunk, 128 gene] bf16 so each
strip is one contiguous 1MB DMA and each [128,128] chunk is a matmul lhsT.
All floating-point math runs on device. X is carried as bf16 hi+lo pairs so
the bf16 matmuls reproduce fp32 accuracy (~1e-6).
"""

import sys

sys.path.insert(0, "/opt/trn_rl_repo")

import numpy as np
import ml_dtypes

import concourse.mybir as mybir
import concourse.tile as tile
from concourse import bacc
from concourse.bass import ts
from concourse.tile import add_dep_helper
from concourse.bass_utils import run_bass_kernel_spmd
from concourse.bass_interp import get_hw_module

F32 = mybir.dt.float32
BF16 = mybir.dt.bfloat16
U8 = mybir.dt.uint8

N_SPOTS = 4096
N_GENES = 20000
D = 128
N_CORES = 8
G_PER = N_GENES // N_CORES  # 2500
P = 128
KCH = N_SPOTS // P  # 32 spot chunks
NX = 258  # [Xhi | Xlo] columns: 2 * (D + 1)


def build_nc(T, wide_mm=False):
    """Build the single-core Bass program (SPMD across 8 cores).

    T = number of 128-gene tiles per core (20 for the real problem).
    wide_mm = one N=258 matmul per chunk (single LDWEIGHTS) instead of two
    N=129 matmuls sharing PSUM columns.
    """
    nc = bacc.Bacc("TRN2", target_bir_lowering=False, debug=False, num_devices=N_CORES)

    maskbt = nc.dram_tensor("maskbt", [T, P, KCH * P], U8, kind="ExternalInput")
    # emb pre-swizzled on host to spot-partition layout: [p, k*128+d] =
    # emb[k*128+p, d] -> each SBUF partition line is one contiguous 16KB read
    embcp = nc.dram_tensor("embcp", [P, KCH * D], F32, kind="ExternalInput")
    embT = nc.dram_tensor("embT", [D, N_SPOTS], F32, kind="ExternalInput")
    wt = nc.dram_tensor("wt", [D, D], F32, kind="ExternalInput")
    bb = nc.dram_tensor("bb", [D, 1], F32, kind="ExternalInput")
    vv = nc.dram_tensor("vv", [D, 1], F32, kind="ExternalInput")
    out = nc.dram_tensor("out", [T, P, D], F32, kind="ExternalOutput")

    with tile.TileContext(nc) as tc:
        with (
            tc.tile_pool(name="const", bufs=1) as constp,
            tc.tile_pool(name="xfp", bufs=1) as xfp,
            tc.tile_pool(name="maskp", bufs=5) as maskp,
            tc.tile_pool(name="outp", bufs=2) as outp,
            tc.tile_pool(name="php", bufs=4, space="PSUM") as php,
            tc.tile_pool(name="pep", bufs=1, space="PSUM") as pep,
            tc.tile_pool(name="ptp", bufs=3, space="PSUM") as ptp,
        ):
            # ---- constants into SBUF ----
            wt_sb = constp.tile([P, D], F32)
            nc.sync.dma_start(out=wt_sb[:], in_=wt[:])
            b_sb = constp.tile([P, 1], F32)
            nc.sync.dma_start(out=b_sb[:], in_=bb[:])
            v_sb = constp.tile([P, 1], F32)
            nc.sync.dma_start(out=v_sb[:], in_=vv[:])
            # big loads split in halves across both HWDGE rings so they run
            # in parallel and downstream compute can start on the first half
            HS = N_SPOTS // 2
            embT_sb = constp.tile([P, N_SPOTS], F32)
            embT_dma1 = nc.sync.dma_start(out=embT_sb[:, 0:HS], in_=embT[:, 0:HS])
            embT_dma2 = nc.scalar.dma_start(out=embT_sb[:, HS:], in_=embT[:, HS:])

            neg30 = constp.tile([P, 1], F32)
            nc.gpsimd.memset(neg30[:], -30.0)

            th_sb = constp.tile([P, N_SPOTS], F32)  # tanh(W h + b).T  [j, s]
            e_sb = constp.tile([P, KCH], F32)  # e in spot-partition layout
            xhl = constp.tile([P, KCH * NX], BF16)  # [Xhi | Xlo] per chunk

            # ---- prologue: logits ----
            # h.T [j, s] = (W.T).T @ emb.T ; tanh(+b) fused from PSUM
            for c in range(N_SPOTS // 512):
                ph = php.tile([P, 512], F32)
                nc.tensor.matmul(
                    out=ph[:], lhsT=wt_sb[:], rhs=embT_sb[:, ts(c, 512)],
                    start=True, stop=True,
                )
                nc.scalar.activation(
                    out=th_sb[:, ts(c, 512)], in_=ph[:],
                    func=mybir.ActivationFunctionType.Tanh, bias=b_sb[:, 0:1],
                )
            # logits, transposed into spot-partition layout:
            # logitsT chunk [128 s, 1] = th_chunk[j, s].T @ v
            pe = pep.tile([P, KCH], F32)
            for k in range(KCH):
                nc.tensor.matmul(
                    out=pe[:, k : k + 1], lhsT=th_sb[:, ts(k, P)], rhs=v_sb[:],
                    start=True, stop=True,
                )
            nc.scalar.activation(
                out=e_sb[:], in_=pe[:],
                func=mybir.ActivationFunctionType.Exp, bias=neg30[:, 0:1],
            )

            # ---- X = [e*emb | e] as bf16 hi + lo (full-width batched ops) ----
            # emb in spot-partition layout; scalar HWDGE ring so it runs in
            # parallel with the embT load on the sync ring
            embc_sb = constp.tile([P, KCH * D], F32)
            HC = KCH * D // 2
            embc_dma1 = nc.sync.dma_start(out=embc_sb[:, 0:HC], in_=embcp[:, 0:HC])
            embc_dma2 = nc.scalar.dma_start(out=embc_sb[:, HC:], in_=embcp[:, HC:])
            xf = xfp.tile([P, KCH * D], F32)
            xhl3 = xhl[:].rearrange("p (k n) -> p k n", n=NX)
            xf3 = xf[:].rearrange("p (k d) -> p k d", d=D)
            emb3 = embc_sb[:].rearrange("p (k d) -> p k d", d=D)
            e3 = e_sb[:].rearrange("p k -> p k ()")
            NG = 4  # build X in chunk groups so the main loop starts early
            GS = KCH // NG
            for g in range(NG):
                ks = slice(g * GS, (g + 1) * GS)
                ebc = e3[:, ks, :].to_broadcast([P, GS, D])
                nc.vector.tensor_mul(out=xf3[:, ks, :], in0=emb3[:, ks, :], in1=ebc)
                hi3 = xhl3[:, ks, 0:D]
                nc.scalar.activation(
                    out=hi3, in_=xf3[:, ks, :], func=mybir.ActivationFunctionType.Copy
                )
                nc.vector.tensor_sub(
                    out=xhl3[:, ks, D + 1 : NX - 1], in0=xf3[:, ks, :], in1=hi3
                )
                nc.vector.tensor_copy(out=xhl3[:, ks, D : D + 1], in_=e3[:, ks, :])
                nc.vector.tensor_sub(
                    out=xhl3[:, ks, NX - 1 : NX], in0=e3[:, ks, :],
                    in1=xhl3[:, ks, D : D + 1],
                )

            # ---- main loop: per gene tile ----
            for t in range(T):
                mt = maskp.tile([P, KCH * P], BF16)
                mdma = nc.gpsimd.dma_start(out=mt[:], in_=maskbt[t])  # uint8 -> bf16 cast in DMA
                if t < 4:
                    # keep the prefetch burst from stealing SDMA engines
                    # while the latency-critical emb loads are in flight
                    for dep in (embT_dma1, embT_dma2, embc_dma1, embc_dma2):
                        add_dep_helper(mdma.ins, dep.ins, True, "mask prefetch after emb")
                if wide_mm:
                    # one LDW + one N=258 matmul per chunk; hi/lo halves
                    # summed on DVE afterwards
                    pt = ptp.tile([P, NX], F32)
                    for k in range(KCH):
                        nc.tensor.matmul(
                            out=pt[:], lhsT=mt[:, ts(k, P)], rhs=xhl[:, ts(k, NX)],
                            start=(k == 0), stop=(k == KCH - 1),
                        )
                    lo_sb = outp.tile([P, D + 1], F32)
                    nc.vector.tensor_copy(out=lo_sb[:], in_=pt[:, D + 1 : NX])
                    s_sb = outp.tile([P, D + 1], F32)
                    nc.vector.tensor_add(out=s_sb[:], in0=pt[:, 0 : D + 1], in1=lo_sb[:])
                else:
                    # hi and lo accumulate into the same PSUM columns
                    pt = ptp.tile([P, D + 1], F32)
                    for k in range(KCH):
                        nc.tensor.matmul(
                            out=pt[:], lhsT=mt[:, ts(k, P)],
                            rhs=xhl[:, k * NX : k * NX + (D + 1)],
                            start=(k == 0), stop=False,
                        )
                        nc.tensor.matmul(
                            out=pt[:], lhsT=mt[:, ts(k, P)],
                            rhs=xhl[:, k * NX + (D + 1) : (k + 1) * NX],
                            start=False, stop=(k == KCH - 1),
                        )
                    s_sb = pt
                rmax = outp.tile([P, 1], F32)
                nc.vector.tensor_scalar_max(out=rmax[:], in0=s_sb[:, D : D + 1], scalar1=1e-37)
                rinv = outp.tile([P, 1], F32)
                nc.vector.reciprocal(out=rinv[:], in_=rmax[:])
                o = outp.tile([P, D], F32)
                nc.vector.tensor_scalar_mul(out=o[:], in0=s_sb[:, 0:D], scalar1=rinv[:, 0:1])
                nc.sync.dma_start(out=out[t], in_=o[:])

    nc.compile()
    return nc


def prep_inputs(spot_emb, W, b, v, gene_ids, spot_ids, T):
    """Host marshaling: shared fp32 operands + per-core mask slabs."""
    emb = np.ascontiguousarray(np.asarray(spot_emb, dtype=np.float32))
    W = np.asarray(W, dtype=np.float32)
    b = np.asarray(b, dtype=np.float32)
    v = np.asarray(v, dtype=np.float32)
    gene_ids = np.asarray(gene_ids).astype(np.int64)
    spot_ids = np.asarray(spot_ids).astype(np.int64)

    shared = {
        "embcp": np.ascontiguousarray(
            emb.reshape(KCH, P, D).transpose(1, 0, 2).reshape(P, KCH * D)
        ),
        "embT": np.ascontiguousarray(emb.T),
        "wt": np.ascontiguousarray(W.T),
        "bb": np.ascontiguousarray(b.reshape(D, 1)),
        "vv": np.ascontiguousarray(v.reshape(D, 1)),
    }

    # Dense 0/1 occupancy mask (set semantics: duplicate edges collapse),
    # built directly in the per-core padded layout: core c's genes live at
    # rows [c*T*P, c*T*P + G_PER); rows above G_PER stay zero padding.
    g_pad = T * P
    M = np.zeros((N_CORES * g_pad, N_SPOTS), dtype=bool)
    pad_rows = (gene_ids // G_PER) * g_pad + (gene_ids % G_PER)
    M[pad_rows, spot_ids] = True
    # [c, t*128+g, k*128+p] -> [c, t, p, k, g]
    Mbt = M.reshape(N_CORES, T, P, KCH, P).transpose(0, 1, 4, 3, 2)
    Mbt = np.ascontiguousarray(Mbt).astype(np.uint8).reshape(N_CORES, T, P, KCH * P)
    return [{"maskbt": Mbt[c], **shared} for c in range(N_CORES)]


_NC_CACHE = {}


def run(spot_emb, W, b, v, gene_ids, spot_ids, trace=False, wide_mm=False, **hw_kwargs):
    T = (G_PER + P - 1) // P  # 20
    key = (T, wide_mm)
    if key not in _NC_CACHE:
        nc = build_nc(T, wide_mm=wide_mm)
        nc.m = get_hw_module(nc.m)
        _NC_CACHE[key] = nc
    nc = _NC_CACHE[key]
    in_maps = prep_inputs(spot_emb, W, b, v, gene_ids, spot_ids, T)
    res = run_bass_kernel_spmd(
        nc, in_maps, core_ids=list(range(N_CORES)), trace=trace, **hw_kwargs
    )
    outs = [
        np.asarray(res.results[c]["out"], dtype=np.float32).reshape(T * P, D)[:G_PER]
        for c in range(N_CORES)
    ]
    full = np.concatenate(outs, axis=0)
    return full, res


def kernel(spot_emb, W, b, v, gene_ids, spot_ids, n_genes):
    n_genes = int(n_genes)
    assert n_genes == N_GENES, f"kernel hardcodes n_genes={N_GENES}, got {n_genes}"
    full, _ = run(spot_emb, W, b, v, gene_ids, spot_ids, trace=False)
    return full


# revision 34
# speedup vs baseline: 1.0726x; 1.0044x over previous
"""Trainium2 Bass kernel for nn_AttentionPool (segment softmax-pool over gene/spot edges).

Math: out[g] = (sum_{s in S_g} e_s * emb[s]) / (sum_{s in S_g} e_s),
      e_s = exp(logit_s - 30),  logit = tanh(emb @ W.T + b) @ v
where S_g is the *set* of distinct spots expressing gene g (duplicate edges
count once), and empty genes produce 0. The row-max shift of the reference
softmax cancels; the constant -30 shift keeps exp() in fp32 range
(|logit| <= sum|v| < 27.6 for this problem's xavier init).

Sharding: 2500 genes per core x 8 cores (padded to 2560 = 20 tiles of 128).
Host marshals the edge list into each core's dense 0/1 mask slab, laid out
as [20 gene-tiles, 128 spot-partition, 32 spot-chunk, 128 gene] bf16 so each
strip is one contiguous 1MB DMA and each [128,128] chunk is a matmul lhsT.
All floating-point math runs on device. X is carried as bf16 hi+lo pairs so
the bf16 matmuls reproduce fp32 accuracy (~1e-6).
"""

import sys

sys.path.insert(0, "/opt/trn_rl_repo")

import numpy as np
import ml_dtypes

import concourse.mybir as mybir
import concourse.tile as tile
from concourse import bacc
from concourse.bass import ts
from concourse.tile import add_dep_helper
from concourse.bass_utils import run_bass_kernel_spmd
from concourse.bass_interp import get_hw_module

F32 = mybir.dt.float32
BF16 = mybir.dt.bfloat16
U8 = mybir.dt.uint8

N_SPOTS = 4096
N_GENES = 20000
D = 128
N_CORES = 8
G_PER = N_GENES // N_CORES  # 2500
P = 128
KCH = N_SPOTS // P  # 32 spot chunks
NX = 258  # [Xhi | Xlo] columns: 2 * (D + 1)


def build_nc(T, wide_mm=False):
    """Build the single-core Bass program (SPMD across 8 cores).

    T = number of 128-gene tiles per core (20 for the real problem).
    wide_mm = one N=258 matmul per chunk (single LDWEIGHTS) instead of two
    N=129 matmuls sharing PSUM columns.
    """
    nc = bacc.Bacc("TRN2", target_bir_lowering=False, debug=False, num_devices=N_CORES)

    maskbt = nc.dram_tensor("maskbt", [T, P, KCH * P], U8, kind="ExternalInput")
    # emb pre-swizzled on host to spot-partition layout: [p, k*128+d] =
    # emb[k*128+p, d] -> each SBUF partition line is one contiguous 16KB read
    embcp = nc.dram_tensor("embcp", [P, KCH * D], F32, kind="ExternalInput")
    embT = nc.dram_tensor("embT", [D, N_SPOTS], F32, kind="ExternalInput")
    wt = nc.dram_tensor("wt", [D, D], F32, kind="ExternalInput")
    bb = nc.dram_tensor("bb", [D, 1], F32, kind="ExternalInput")
    vv = nc.dram_tensor("vv", [D, 1], F32, kind="ExternalInput")
    out = nc.dram_tensor("out", [T, P, D], F32, kind="ExternalOutput")

    with tile.TileContext(nc) as tc:
        with (
            tc.tile_pool(name="const", bufs=1) as constp,
            tc.tile_pool(name="xfp", bufs=1) as xfp,
            tc.tile_pool(name="maskp", bufs=5) as maskp,
            tc.tile_pool(name="outp", bufs=2) as outp,
            tc.tile_pool(name="php", bufs=4, space="PSUM") as php,
            tc.tile_pool(name="pep", bufs=1, space="PSUM") as pep,
            tc.tile_pool(name="ptp", bufs=3, space="PSUM") as ptp,
        ):
            # ---- constants into SBUF ----
            wt_sb = constp.tile([P, D], F32)
            nc.sync.dma_start(out=wt_sb[:], in_=wt[:])
            b_sb = constp.tile([P, 1], F32)
            nc.sync.dma_start(out=b_sb[:], in_=bb[:])
            v_sb = constp.tile([P, 1], F32)
            nc.sync.dma_start(out=v_sb[:], in_=vv[:])
            # big loads split in halves across both HWDGE rings, each half its
            # own tile so downstream compute starts as soon as its half lands
            HS = N_SPOTS // 2
            embT_a = constp.tile([P, HS], F32)
            embT_b = constp.tile([P, HS], F32)
            embT_dma1 = nc.sync.dma_start(out=embT_a[:], in_=embT[:, 0:HS])
            embT_dma2 = nc.scalar.dma_start(out=embT_b[:], in_=embT[:, HS:])

            def embT_cols(lo, width):
                # view into the correct half-tile (never straddles: callers
                # use 512- or 128-aligned slices within one half)
                if lo < HS:
                    return embT_a[:, lo : lo + width]
                return embT_b[:, lo - HS : lo - HS + width]

            neg30 = constp.tile([P, 1], F32)
            nc.gpsimd.memset(neg30[:], -30.0)

            th_sb = constp.tile([P, N_SPOTS], F32)  # tanh(W h + b).T  [j, s]
            e_sb = constp.tile([P, KCH], F32)  # e in spot-partition layout
            xhl = constp.tile([P, KCH * NX], BF16)  # [Xhi | Xlo] per chunk

            # ---- prologue: logits ----
            # h.T [j, s] = (W.T).T @ emb.T ; tanh(+b) fused from PSUM
            for c in range(N_SPOTS // 512):
                ph = php.tile([P, 512], F32)
                nc.tensor.matmul(
                    out=ph[:], lhsT=wt_sb[:], rhs=embT_cols(c * 512, 512),
                    start=True, stop=True,
                )
                nc.scalar.activation(
                    out=th_sb[:, ts(c, 512)], in_=ph[:],
                    func=mybir.ActivationFunctionType.Tanh, bias=b_sb[:, 0:1],
                )
            # logits, transposed into spot-partition layout:
            # logitsT chunk [128 s, 1] = th_chunk[j, s].T @ v
            pe = pep.tile([P, KCH], F32)
            for k in range(KCH):
                nc.tensor.matmul(
                    out=pe[:, k : k + 1], lhsT=th_sb[:, ts(k, P)], rhs=v_sb[:],
                    start=True, stop=True,
                )
            nc.scalar.activation(
                out=e_sb[:], in_=pe[:],
                func=mybir.ActivationFunctionType.Exp, bias=neg30[:, 0:1],
            )

            # ---- X = [e*emb | e] as bf16 hi + lo (full-width batched ops) ----
            # emb in spot-partition layout; scalar HWDGE ring so it runs in
            # parallel with the embT load on the sync ring
            HC = KCH * D // 2
            embc_a = constp.tile([P, HC], F32)
            embc_b = constp.tile([P, HC], F32)
            embc_dma1 = nc.sync.dma_start(out=embc_a[:], in_=embcp[:, 0:HC])
            embc_dma2 = nc.scalar.dma_start(out=embc_b[:], in_=embcp[:, HC:])
            xf = xfp.tile([P, KCH * D], F32)
            xhl3 = xhl[:].rearrange("p (k n) -> p k n", n=NX)
            xf3 = xf[:].rearrange("p (k d) -> p k d", d=D)
            emb3a = embc_a[:].rearrange("p (k d) -> p k d", d=D)
            emb3b = embc_b[:].rearrange("p (k d) -> p k d", d=D)
            e3 = e_sb[:].rearrange("p k -> p k ()")
            NG = 4  # build X in chunk groups so the main loop starts early
            GS = KCH // NG
            for g in range(NG):
                ks = slice(g * GS, (g + 1) * GS)
                if g < NG // 2:
                    embsrc = emb3a[:, ks, :]
                else:
                    embsrc = emb3b[:, slice(g * GS - KCH // 2, (g + 1) * GS - KCH // 2), :]
                ebc = e3[:, ks, :].to_broadcast([P, GS, D])
                nc.vector.tensor_mul(out=xf3[:, ks, :], in0=embsrc, in1=ebc)
                hi3 = xhl3[:, ks, 0:D]
                nc.scalar.activation(
                    out=hi3, in_=xf3[:, ks, :], func=mybir.ActivationFunctionType.Copy
                )
                nc.vector.tensor_sub(
                    out=xhl3[:, ks, D + 1 : NX - 1], in0=xf3[:, ks, :], in1=hi3
                )
                nc.vector.tensor_copy(out=xhl3[:, ks, D : D + 1], in_=e3[:, ks, :])
                nc.vector.tensor_sub(
                    out=xhl3[:, ks, NX - 1 : NX], in0=e3[:, ks, :],
                    in1=xhl3[:, ks, D : D + 1],
                )

            # ---- main loop: per gene tile ----
            for t in range(T):
                mt = maskp.tile([P, KCH * P], BF16, name=f"mt{t}", tag="mt")
                mdma = nc.gpsimd.dma_start(out=mt[:], in_=maskbt[t])  # u8->bf16 cast
                if t < 4:
                    # keep the prefetch burst from stealing SDMA engines
                    # while the latency-critical emb loads are in flight
                    for dep in (embT_dma1, embT_dma2, embc_dma1, embc_dma2):
                        add_dep_helper(mdma.ins, dep.ins, True, "mask after emb")
                if wide_mm:
                    # one LDW + one N=258 matmul per chunk; hi/lo halves
                    # summed on DVE afterwards
                    pt = ptp.tile([P, NX], F32, name=f"ptw{t}", tag="pt")
                    for k in range(KCH):
                        nc.tensor.matmul(
                            out=pt[:], lhsT=mt[:, ts(k, P)], rhs=xhl[:, ts(k, NX)],
                            start=(k == 0), stop=(k == KCH - 1),
                        )
                    lo_sb = outp.tile([P, D + 1], F32, tag="lo_sb")
                    nc.vector.tensor_copy(out=lo_sb[:], in_=pt[:, D + 1 : NX])
                    s_sb = outp.tile([P, D + 1], F32, tag="s_sb")
                    nc.vector.tensor_add(out=s_sb[:], in0=pt[:, 0 : D + 1], in1=lo_sb[:])
                else:
                    # hi and lo accumulate into the same PSUM columns
                    pt = ptp.tile([P, D + 1], F32, name=f"ptn{t}", tag="pt")
                    for k in range(KCH):
                        nc.tensor.matmul(
                            out=pt[:], lhsT=mt[:, ts(k, P)],
                            rhs=xhl[:, k * NX : k * NX + (D + 1)],
                            start=(k == 0), stop=False,
                        )
                        nc.tensor.matmul(
                            out=pt[:], lhsT=mt[:, ts(k, P)],
                            rhs=xhl[:, k * NX + (D + 1) : (k + 1) * NX],
                            start=False, stop=(k == KCH - 1),
                        )
                    s_sb = pt
                rmax = outp.tile([P, 1], F32, tag="rmax")
                nc.vector.tensor_scalar_max(out=rmax[:], in0=s_sb[:, D : D + 1], scalar1=1e-37)
                rinv = outp.tile([P, 1], F32, tag="rinv")
                nc.vector.reciprocal(out=rinv[:], in_=rmax[:])
                o = outp.tile([P, D], F32, tag="o")
                nc.vector.tensor_scalar_mul(out=o[:], in0=s_sb[:, 0:D], scalar1=rinv[:, 0:1])
                nc.sync.dma_start(out=out[t], in_=o[:])

    nc.compile()
    return nc


def prep_inputs(spot_emb, W, b, v, gene_ids, spot_ids, T):
    """Host marshaling: shared fp32 operands + per-core mask slabs."""
    emb = np.ascontiguousarray(np.asarray(spot_emb, dtype=np.float32))
    W = np.asarray(W, dtype=np.float32)
    b = np.asarray(b, dtype=np.float32)
    v = np.asarray(v, dtype=np.float32)
    gene_ids = np.asarray(gene_ids).astype(np.int64)
    spot_ids = np.asarray(spot_ids).astype(np.int64)

    shared = {
        "embcp": np.ascontiguousarray(
            emb.reshape(KCH, P, D).transpose(1, 0, 2).reshape(P, KCH * D)
        ),
        "embT": np.ascontiguousarray(emb.T),
        "wt": np.ascontiguousarray(W.T),
        "bb": np.ascontiguousarray(b.reshape(D, 1)),
        "vv": np.ascontiguousarray(v.reshape(D, 1)),
    }

    # Dense 0/1 occupancy mask (set semantics: duplicate edges collapse),
    # built directly in the per-core padded layout: core c's genes live at
    # rows [c*T*P, c*T*P + G_PER); rows above G_PER stay zero padding.
    g_pad = T * P
    M = np.zeros((N_CORES * g_pad, N_SPOTS), dtype=bool)
    pad_rows = (gene_ids // G_PER) * g_pad + (gene_ids % G_PER)
    M[pad_rows, spot_ids] = True
    # [c, t*128+g, k*128+p] -> [c, t, p, k, g]
    Mbt = M.reshape(N_CORES, T, P, KCH, P).transpose(0, 1, 4, 3, 2)
    Mbt = np.ascontiguousarray(Mbt).astype(np.uint8).reshape(N_CORES, T, P, KCH * P)
    return [{"maskbt": Mbt[c], **shared} for c in range(N_CORES)]


_NC_CACHE = {}


def run(spot_emb, W, b, v, gene_ids, spot_ids, trace=False, wide_mm=False, **hw_kwargs):
    T = (G_PER + P - 1) // P  # 20
    key = (T, wide_mm)
    if key not in _NC_CACHE:
        nc = build_nc(T, wide_mm=wide_mm)
        nc.m = get_hw_module(nc.m)
        _NC_CACHE[key] = nc
    nc = _NC_CACHE[key]
    in_maps = prep_inputs(spot_emb, W, b, v, gene_ids, spot_ids, T)
    res = run_bass_kernel_spmd(
        nc, in_maps, core_ids=list(range(N_CORES)), trace=trace, **hw_kwargs
    )
    outs = [
        np.asarray(res.results[c]["out"], dtype=np.float32).reshape(T * P, D)[:G_PER]
        for c in range(N_CORES)
    ]
    full = np.concatenate(outs, axis=0)
    return full, res


def kernel(spot_emb, W, b, v, gene_ids, spot_ids, n_genes):
    n_genes = int(n_genes)
    assert n_genes == N_GENES, f"kernel hardcodes n_genes={N_GENES}, got {n_genes}"
    full, _ = run(spot_emb, W, b, v, gene_ids, spot_ids, trace=False)
    return full
